# revision 2
# baseline (speedup 1.0000x reference)
"""AlignedTripletLoss Trainium2 kernel (8 NeuronCores, data-parallel over anchors).

Math (matches reference.py):
  x_hat = x / (||x||_2 + 1e-12) per (image, part) row               [1024*8, 128]
  dist2[(a,i),(b,j)] = 2 - 2 * <x_hat_(a,i), x_hat_(b,j)>  (rows are unit norm,
      so the sq-norm terms are 1 up to ~1e-6; a +1e-5 bias keeps sqrt's argument
      positive on the diagonal, adding <3e-6 relative error elsewhere)
  t = tanh(0.5 * sqrt(dist2))
  dtw[a,b] = monotone (right/down) shortest path over the 8x8 grid t[i][j]
  ap = max over positives, an = min over negatives, loss = mean(relu(ap-an+0.3))

Per-core: 128 anchor rows x 1024 cols. The DTW row recurrence
  val[j] = min(val[j-1], up[j]) + t[i][j]
is exactly DVE tensor_tensor_scan(op0=min, op1=add) along the free axis, with a
dummy element between consecutive (a,b) pairs to reset the running state.
"""

import numpy as np

N, M, D = 1024, 8, 128
MARGIN = 0.3
EPS = 1e-12
NCORES = 8
A = N // NCORES          # anchors per core
CB = 256                 # column-batch size
NB = N // CB             # batches
G = M + 1                # scan group: 1 dummy + 8 j-steps
BIG = 1e9
SQ_BIAS = 2.0 + 1e-5

_CACHE = {}


def _build_nc():
    import concourse.bacc as bacc
    import concourse.mybir as mybir
    import concourse.tile as tile
    from concourse.tile import add_dep_helper
    from concourse.masks import make_identity

    fp32 = mybir.dt.float32
    AF = mybir.ActivationFunctionType
    OP = mybir.AluOpType
    AX = mybir.AxisListType

    nc = bacc.Bacc("TRN2", target_bir_lowering=False, debug=False,
                   num_devices=NCORES)

    x_in = nc.dram_tensor("x", [N * M, D], fp32, kind="ExternalInput")
    xa_in = nc.dram_tensor("xa", [A * M, D], fp32, kind="ExternalInput")
    pm_in = nc.dram_tensor("posmask", [A, N], fp32, kind="ExternalInput")
    nm_in = nc.dram_tensor("negmask", [A, N], fp32, kind="ExternalInput")
    out_t = nc.dram_tensor("row_loss", [A, 1], fp32, kind="ExternalOutput")

    S = (N * M) // 128   # 64 row-tiles of x
    SA = (A * M) // 128  # 8 row-tiles of xa

    with tile.TileContext(nc) as tc:
        with tc.tile_pool(name="persist", bufs=1) as persist:
            xT = persist.tile([128, M, N], fp32)    # x_hat^T, [d][j][b]
            xTa = persist.tile([128, M, A], fp32)   # -2*x_hat_anchor^T, [d][i][a]
            pmask = persist.tile([128, N], fp32)
            nmask = persist.tile([128, N], fp32)
            up0 = persist.tile([128, CB * G], fp32)
            biasT = persist.tile([128, 1], fp32)
            apacc = persist.tile([128, NB], fp32)
            anacc = persist.tile([128, NB], fp32)

            nc.sync.dma_start(pmask[:], pm_in[:])
            nc.sync.dma_start(nmask[:], nm_in[:])
            nc.gpsimd.memset(biasT[:], SQ_BIAS)
            up0v = up0.rearrange("p (c g) -> p c g", g=G)
            nc.gpsimd.memset(up0v[:, :, 0:1], -BIG)
            nc.gpsimd.memset(up0v[:, :, 1:G], BIG)

            # ---------- setup: normalize + transpose ----------
            with (
                tc.tile_pool(name="setup", bufs=1) as setup,
                tc.tile_pool(name="tpsum", bufs=2, space="PSUM") as tpsum,
            ):
                ident = setup.tile([128, 128], fp32)
                make_identity(nc, ident[:])

                def norm_rows(src_dram, n_tiles, neg2):
                    # rows laid out p-outer: row r = p*n_tiles + s
                    xr = setup.tile([128, n_tiles, D], fp32, tag=f"xr{n_tiles}")
                    nc.sync.dma_start(
                        xr[:], src_dram.rearrange("(p s) d -> p s d", p=128))
                    x2 = setup.tile([128, n_tiles, D], fp32, tag=f"x2{n_tiles}")
                    nc.scalar.activation(x2[:], xr[:], AF.Square)
                    n2 = setup.tile([128, n_tiles], fp32, tag=f"n2{n_tiles}")
                    nc.vector.tensor_reduce(n2[:], x2[:], axis=AX.X, op=OP.add)
                    nrm = setup.tile([128, n_tiles], fp32, tag=f"nr{n_tiles}")
                    nc.scalar.activation(nrm[:], n2[:], AF.Sqrt)
                    nc.vector.tensor_scalar_add(nrm[:], nrm[:], EPS)
                    rn = setup.tile([128, n_tiles], fp32, tag=f"rn{n_tiles}")
                    nc.vector.reciprocal(rn[:], nrm[:])
                    if neg2:
                        nc.vector.tensor_scalar_mul(rn[:], rn[:], -2.0)
                    xn = setup.tile([128, n_tiles, D], fp32, tag=f"xn{n_tiles}")
                    for s in range(n_tiles):
                        nc.vector.tensor_scalar_mul(
                            xn[:, s, :], xr[:, s, :], rn[:, s:s + 1])
                    return xn

                xn = norm_rows(x_in, S, neg2=False)
                for s in range(S):
                    pt = tpsum.tile([128, 128], fp32, tag="tp")
                    nc.tensor.transpose(pt[:], xn[:, s, :], ident[:])
                    # column p of pt = global row p*64+s = (b=8p+s//8, j=s%8)
                    dst = xT[:, s % M, (s // M)::M]
                    if s % 2 == 0:
                        nc.vector.tensor_copy(dst, pt[:])
                    else:
                        nc.scalar.copy(dst, pt[:])

                xna = norm_rows(xa_in, SA, neg2=True)
                for s in range(SA):
                    pt = tpsum.tile([128, 128], fp32, tag="tp")
                    nc.tensor.transpose(pt[:], xna[:, s, :], ident[:])
                    # row r = p*8+s -> (a=p, i=s)
                    nc.vector.tensor_copy(xTa[:, s, :], pt[:])

            # ---------- main loop ----------
            with (
                tc.tile_pool(name="rows", bufs=10) as rows,
                tc.tile_pool(name="valsp", bufs=3) as valsp,
                tc.tile_pool(name="mtmp", bufs=6) as mtmp,
                tc.tile_pool(name="mpsum", bufs=2, space="PSUM") as mpsum,
            ):
                prev_tanh_last = None
                for n in range(NB):
                    sd = []
                    sqrt_insts = []
                    for i in range(M):
                        pp = mpsum.tile([128, M, CB], fp32, tag="pp")
                        for j in range(M):
                            nc.tensor.matmul(
                                pp[:, j, :], lhsT=xTa[:, i, :],
                                rhs=xT[:, j, n * CB:(n + 1) * CB],
                                start=True, stop=True)
                        buf = rows.tile([128, CB, G], fp32, tag="row")
                        nc.gpsimd.memset(buf[:, :, 0:1], BIG)
                        sq_out = buf[:, :, 1:G].rearrange("p b j -> p j b")
                        inst = nc.scalar.activation(
                            sq_out, pp[:], AF.Sqrt, bias=biasT[:, 0:1])
                        if prev_tanh_last is not None:
                            add_dep_helper(inst.ins, prev_tanh_last.ins, sync=False,
                                           reason="ACT table batch order")
                        sqrt_insts.append(inst)
                        sd.append(buf)
                    tanh_last = None
                    for i in range(M):
                        v = sd[i][:, :, 1:G]
                        t_inst = nc.scalar.activation(v, v, AF.Tanh, scale=0.5)
                        add_dep_helper(t_inst.ins, sqrt_insts[-1].ins, sync=False,
                                       reason="ACT table batch order")
                        tanh_last = t_inst
                    prev_tanh_last = tanh_last
                    prev_vals = None
                    for i in range(M):
                        vt = valsp.tile([128, CB * G], fp32, tag="vals")
                        d0 = up0[:] if i == 0 else prev_vals[:]
                        nc.vector.tensor_tensor_scan(
                            vt[:], d0, sd[i].rearrange("p c g -> p (c g)"),
                            0.0, OP.min, OP.add)
                        prev_vals = vt
                    dtw = prev_vals.rearrange("p (c g) -> p c g", g=G)[:, :, M:M + 1]
                    dtw = dtw.rearrange("p c o -> p (c o)")
                    tp = mtmp.tile([128, CB], fp32, tag="tp")
                    nc.vector.tensor_tensor(
                        tp[:], dtw, pmask[:, n * CB:(n + 1) * CB], OP.add)
                    nc.vector.tensor_reduce(
                        apacc[:, n:n + 1], tp[:], axis=AX.X, op=OP.max)
                    tn = mtmp.tile([128, CB], fp32, tag="tn")
                    nc.vector.tensor_tensor(
                        tn[:], dtw, nmask[:, n * CB:(n + 1) * CB], OP.add)
                    nc.vector.tensor_reduce(
                        anacc[:, n:n + 1], tn[:], axis=AX.X, op=OP.min)

                ap = mtmp.tile([128, 1], fp32, tag="fin")
                an = mtmp.tile([128, 1], fp32, tag="fin2")
                nc.vector.tensor_reduce(ap[:], apacc[:], axis=AX.X, op=OP.max)
                nc.vector.tensor_reduce(an[:], anacc[:], axis=AX.X, op=OP.min)
                lv = mtmp.tile([128, 1], fp32, tag="fin3")
                nc.vector.tensor_tensor(lv[:], ap[:], an[:], OP.subtract)
                nc.vector.tensor_scalar(
                    lv[:], lv[:], MARGIN, 0.0, OP.add, OP.max)
                nc.sync.dma_start(out_t[:], lv[:])

    nc.compile()
    return nc


def _get_nc():
    if "nc" not in _CACHE:
        _CACHE["nc"] = _build_nc()
    return _CACHE["nc"]


def kernel(inputs, labels, _trace=False, _trace_cores=None):
    from concourse.bass_utils import run_bass_kernel_spmd

    x = np.ascontiguousarray(np.asarray(inputs, dtype=np.float32)).reshape(N * M, D)
    lab = np.asarray(labels)
    eq = lab[:, None] == lab[None, :]
    posmask = np.where(eq, np.float32(0.0), np.float32(-1e30)).astype(np.float32)
    negmask = np.where(eq, np.float32(1e30), np.float32(0.0)).astype(np.float32)

    nc = _get_nc()
    in_maps = []
    for c in range(NCORES):
        a0 = c * A
        in_maps.append({
            "x": x,
            "xa": np.ascontiguousarray(x[a0 * M:(a0 + A) * M]),
            "posmask": np.ascontiguousarray(posmask[a0:a0 + A]),
            "negmask": np.ascontiguousarray(negmask[a0:a0 + A]),
        })
    res = run_bass_kernel_spmd(
        nc, in_maps, core_ids=list(range(NCORES)), trace=_trace,
        trace_cores=_trace_cores)
    if _trace:
        _CACHE["last_results"] = res
    row_loss = np.concatenate([r["row_loss"][:, 0] for r in res.results])
    return np.asarray(row_loss.mean(), dtype=np.float32)


# revision 4
# speedup vs baseline: 1.4926x; 1.4926x over previous
"""AlignedTripletLoss Trainium2 kernel (8 NeuronCores, data-parallel over anchors).

Math (matches reference.py):
  x_hat = x / (||x||_2 + 1e-12) per (image, part) row               [1024*8, 128]
  dist2[(a,i),(b,j)] = 2 - 2 * <x_hat_(a,i), x_hat_(b,j)>  (rows are unit norm,
      so the sq-norm terms are 1 up to ~1e-6; a +4e-4 bias keeps sqrt's argument
      positive on the diagonal despite float32r matmul rounding, adding ~1e-4
      relative error elsewhere -- far below the loss tolerance)
  t = tanh(0.5 * sqrt(dist2))
  dtw[a,b] = monotone (right/down) shortest path over the 8x8 grid t[i][j]
  ap = max over positives, an = min over negatives, loss = mean(relu(ap-an+0.3))

Per-core: 128 anchor rows x 1024 cols. The DTW row recurrence
  val[j] = min(val[j-1], up[j]) + t[i][j]
is exactly DVE tensor_tensor_scan(op0=min, op1=add) along the free axis, with a
dummy element between consecutive (a,b) pairs to reset the running state.

Perf notes:
 - pairwise dots run as float32r matmuls (1 cyc/col at free dim >= 256 vs 4 for
   fp32); inputs are rounded to f32r by the producing copies as walrus requires.
 - normalization scale is folded into the transpose: PE matmul against a
   gpsimd-built diag(1/norm) both transposes and scales in one pass.
 - sqrt reads PSUM with a scattered (b,j) AP (free) and writes the scan layout
   in 32B runs; the scattered-write ordering costs 2x (measured).
"""

import numpy as np

N, M, D = 1024, 8, 128
MARGIN = 0.3
EPS = 1e-12
NCORES = 8
A = N // NCORES          # anchors per core
CB = 256                 # column-batch size
NB = N // CB             # batches
G = M + 1                # scan group: 1 dummy + 8 j-steps
BIG = 1e9
SQ_BIAS = 2.0 + 4e-4

_CACHE = {}


def _build_nc():
    import concourse.bacc as bacc
    import concourse.mybir as mybir
    import concourse.tile as tile
    from concourse.tile import add_dep_helper

    fp32 = mybir.dt.float32
    f32r = mybir.dt.float32r
    AF = mybir.ActivationFunctionType
    OP = mybir.AluOpType
    AX = mybir.AxisListType

    nc = bacc.Bacc("TRN2", target_bir_lowering=False, debug=False,
                   num_devices=NCORES)

    x_in = nc.dram_tensor("x", [N * M, D], fp32, kind="ExternalInput")
    xa_in = nc.dram_tensor("xa", [A * M, D], fp32, kind="ExternalInput")
    pm_in = nc.dram_tensor("posmask", [A, N], fp32, kind="ExternalInput")
    nm_in = nc.dram_tensor("negmask", [A, N], fp32, kind="ExternalInput")
    out_t = nc.dram_tensor("row_loss", [A, 1], fp32, kind="ExternalOutput")

    S = (N * M) // 128   # 64 row-tiles of x
    SA = (A * M) // 128  # 8 row-tiles of xa

    with tile.TileContext(nc) as tc:
        with tc.tile_pool(name="persist", bufs=1) as persist:
            xT = persist.tile([128, N, M], f32r)    # x_hat^T, [d][b][j] (b-major)
            xTa = persist.tile([128, M, A], f32r)   # -2*x_hat_anchor^T, [d][i][a]
            pmask = persist.tile([128, N], fp32)
            nmask = persist.tile([128, N], fp32)
            up0 = persist.tile([128, CB * G], fp32)
            biasT = persist.tile([128, 1], fp32)
            apacc = persist.tile([128, NB], fp32)
            anacc = persist.tile([128, NB], fp32)

            nc.sync.dma_start(pmask[:], pm_in[:])
            nc.sync.dma_start(nmask[:], nm_in[:])
            nc.gpsimd.memset(biasT[:], SQ_BIAS)
            up0v = up0.rearrange("p (c g) -> p c g", g=G)
            nc.gpsimd.memset(up0v[:, :, 0:1], -BIG)
            nc.gpsimd.memset(up0v[:, :, 1:G], BIG)

            # ---------- setup: normalize + transpose (scale fused via diag) ----
            with (
                tc.tile_pool(name="setup", bufs=1) as setup,
                tc.tile_pool(name="tpsum", bufs=2, space="PSUM") as tpsum,
            ):
                def norm_diag(src_dram, n_tiles, neg2, tagp):
                    """Load rows (p-outer), compute rn=1/(||row||+eps), build
                    per-tile diag(rn) matrices on gpsimd. Returns (xr, diags)."""
                    xr = setup.tile([128, n_tiles, D], fp32, tag=f"xr{tagp}")
                    nc.sync.dma_start(
                        xr[:], src_dram.rearrange("(p s) d -> p s d", p=128))
                    x2 = setup.tile([128, n_tiles, D], fp32, tag=f"x2{tagp}")
                    nc.scalar.activation(x2[:], xr[:], AF.Square)
                    n2 = setup.tile([128, n_tiles], fp32, tag=f"n2{tagp}")
                    nc.vector.tensor_reduce(n2[:], x2[:], axis=AX.X, op=OP.add)
                    nrm = setup.tile([128, n_tiles], fp32, tag=f"nr{tagp}")
                    nc.scalar.activation(nrm[:], n2[:], AF.Sqrt)
                    nc.vector.tensor_scalar_add(nrm[:], nrm[:], EPS)
                    rn = setup.tile([128, n_tiles], fp32, tag=f"rn{tagp}")
                    nc.vector.reciprocal(rn[:], nrm[:])
                    if neg2:
                        nc.vector.tensor_scalar_mul(rn[:], rn[:], -2.0)
                    diags = setup.tile([128, n_tiles, 128], fp32, tag=f"dg{tagp}")
                    for s in range(n_tiles):
                        # dg[p, q] = rn[p, s] where p == q else 0
                        nc.gpsimd.affine_select(
                            out=diags[:, s, :],
                            in_=rn[:, s:s + 1].to_broadcast((128, 128)),
                            compare_op=OP.is_equal, fill=0.0, base=0,
                            pattern=[[-1, 128]], channel_multiplier=1)
                    return xr, diags

                xr, diags = norm_diag(x_in, S, neg2=False, tagp="x")
                # tiles s = 8q + j hold rows r = p*64 + 8q + j -> (b = 8p+q, j)
                for q in range(M):
                    for half in range(2):
                        pt = tpsum.tile([128, 4, 128], fp32, tag="tp")
                        for jj in range(4):
                            s = 8 * q + 4 * half + jj
                            nc.tensor.matmul(
                                pt[:, jj, :], lhsT=xr[:, s, :],
                                rhs=diags[:, s, :], start=True, stop=True)
                        # dst element (j, p) -> xT[:, 8p+q, j]: runs of 4 f32
                        dst = xT[:, q::M, 4 * half:4 * half + 4]
                        nc.scalar.activation(
                            dst.rearrange("d b j -> d j b"), pt[:], AF.Copy)

                xra, diaga = norm_diag(xa_in, SA, neg2=True, tagp="a")
                for half in range(2):
                    pt = tpsum.tile([128, 4, 128], fp32, tag="tp")
                    for jj in range(4):
                        s = 4 * half + jj
                        nc.tensor.matmul(
                            pt[:, jj, :], lhsT=xra[:, s, :],
                            rhs=diaga[:, s, :], start=True, stop=True)
                    # tile s holds rows r = p*8+s -> (a=p, i=s)
                    dst = xTa[:, 4 * half:4 * half + 4, :]
                    nc.scalar.activation(
                        dst.rearrange("d i a -> d i a"), pt[:], AF.Copy)

            # ---------- main loop ----------
            with (
                tc.tile_pool(name="rows", bufs=10) as rows,
                tc.tile_pool(name="valsp", bufs=3) as valsp,
                tc.tile_pool(name="mtmp", bufs=6) as mtmp,
                tc.tile_pool(name="mpsum", bufs=2, space="PSUM") as mpsum,
            ):
                prev_tanh_last = None
                for n in range(NB):
                    sd = []
                    sqrt_insts = []
                    for i in range(M):
                        pp = mpsum.tile([128, M, CB], fp32, tag="pp")
                        for j in range(M):
                            nc.tensor.matmul(
                                pp[:, j, :], lhsT=xTa[:, i, :],
                                rhs=xT[:, n * CB:(n + 1) * CB, j],
                                start=True, stop=True)
                        buf = rows.tile([128, CB, G], fp32, tag="row")
                        nc.gpsimd.memset(buf[:, :, 0:1], BIG)
                        # read PSUM scattered in (b, j) order; write 32B runs
                        inst = nc.scalar.activation(
                            buf[:, :, 1:G], pp.rearrange("p j b -> p b j"),
                            AF.Sqrt, bias=biasT[:, 0:1])
                        if prev_tanh_last is not None:
                            add_dep_helper(inst.ins, prev_tanh_last.ins,
                                           sync=False,
                                           reason="ACT table batch order")
                        sqrt_insts.append(inst)
                        sd.append(buf)
                    tanh_last = None
                    for i in range(M):
                        v = sd[i][:, :, 1:G]
                        t_inst = nc.scalar.activation(v, v, AF.Tanh, scale=0.5)
                        add_dep_helper(t_inst.ins, sqrt_insts[-1].ins,
                                       sync=False,
                                       reason="ACT table batch order")
                        tanh_last = t_inst
                    prev_tanh_last = tanh_last
                    prev_vals = None
                    for i in range(M):
                        vt = valsp.tile([128, CB * G], fp32, tag="vals")
                        d0 = up0[:] if i == 0 else prev_vals[:]
                        nc.vector.tensor_tensor_scan(
                            vt[:], d0, sd[i].rearrange("p c g -> p (c g)"),
                            0.0, OP.min, OP.add)
                        prev_vals = vt
                    dtw = prev_vals.rearrange("p (c g) -> p c g", g=G)[:, :, M:M + 1]
                    dtw = dtw.rearrange("p c o -> p (c o)")
                    tp = mtmp.tile([128, CB], fp32, tag="tp")
                    nc.vector.tensor_tensor(
                        tp[:], dtw, pmask[:, n * CB:(n + 1) * CB], OP.add)
                    nc.vector.tensor_reduce(
                        apacc[:, n:n + 1], tp[:], axis=AX.X, op=OP.max)
                    tn = mtmp.tile([128, CB], fp32, tag="tn")
                    nc.vector.tensor_tensor(
                        tn[:], dtw, nmask[:, n * CB:(n + 1) * CB], OP.add)
                    nc.vector.tensor_reduce(
                        anacc[:, n:n + 1], tn[:], axis=AX.X, op=OP.min)

                ap = mtmp.tile([128, 1], fp32, tag="fin")
                an = mtmp.tile([128, 1], fp32, tag="fin2")
                nc.vector.tensor_reduce(ap[:], apacc[:], axis=AX.X, op=OP.max)
                nc.vector.tensor_reduce(an[:], anacc[:], axis=AX.X, op=OP.min)
                lv = mtmp.tile([128, 1], fp32, tag="fin3")
                nc.vector.tensor_tensor(lv[:], ap[:], an[:], OP.subtract)
                nc.vector.tensor_scalar(
                    lv[:], lv[:], MARGIN, 0.0, OP.add, OP.max)
                nc.sync.dma_start(out_t[:], lv[:])

    nc.compile()
    return nc


def _get_nc():
    if "nc" not in _CACHE:
        _CACHE["nc"] = _build_nc()
    return _CACHE["nc"]


def kernel(inputs, labels, _trace=False, _trace_cores=None):
    from concourse.bass_utils import run_bass_kernel_spmd

    x = np.ascontiguousarray(np.asarray(inputs, dtype=np.float32)).reshape(N * M, D)
    lab = np.asarray(labels)
    eq = lab[:, None] == lab[None, :]
    posmask = np.where(eq, np.float32(0.0), np.float32(-1e30)).astype(np.float32)
    negmask = np.where(eq, np.float32(1e30), np.float32(0.0)).astype(np.float32)

    nc = _get_nc()
    in_maps = []
    for c in range(NCORES):
        a0 = c * A
        in_maps.append({
            "x": x,
            "xa": np.ascontiguousarray(x[a0 * M:(a0 + A) * M]),
            "posmask": np.ascontiguousarray(posmask[a0:a0 + A]),
            "negmask": np.ascontiguousarray(negmask[a0:a0 + A]),
        })
    res = run_bass_kernel_spmd(
        nc, in_maps, core_ids=list(range(NCORES)), trace=_trace,
        trace_cores=_trace_cores)
    if _trace:
        _CACHE["last_results"] = res
    row_loss = np.concatenate([r["row_loss"][:, 0] for r in res.results])
    return np.asarray(row_loss.mean(), dtype=np.float32)


# revision 7
# speedup vs baseline: 1.5257x; 1.0222x over previous
"""AlignedTripletLoss Trainium2 kernel (8 NeuronCores, data-parallel over anchors).

Math (matches reference.py):
  x_hat = x / (||x||_2 + 1e-12) per (image, part) row               [1024*8, 128]
  dist2[(a,i),(b,j)] = 2 - 2 * <x_hat_(a,i), x_hat_(b,j)>  (rows are unit norm,
      so the sq-norm terms are 1 up to ~1e-6; a +4e-4 bias keeps sqrt's argument
      positive on the diagonal despite float32r matmul rounding, adding ~1e-4
      relative error elsewhere -- far below the loss tolerance)
  t = tanh(0.5 * sqrt(dist2))
  dtw[a,b] = monotone (right/down) shortest path over the 8x8 grid t[i][j]
  ap = max over positives, an = min over negatives, loss = mean(relu(ap-an+0.3))

Per-core: 128 anchor rows x 1024 cols. The DTW row recurrence
  val[j] = min(val[j-1], up[j]) + t[i][j]
is exactly DVE tensor_tensor_scan(op0=min, op1=add) along the free axis, with a
dummy element between consecutive (a,b) pairs to reset the running state.

Perf notes:
 - pairwise dots run as float32r matmuls (1 cyc/col at free dim >= 256 vs 4 for
   fp32); inputs are rounded to f32r by the producing copies as walrus requires.
 - normalization scale is folded into the transpose: PE matmul against a
   gpsimd-built diag(1/norm) both transposes and scales in one pass.
 - sqrt reads PSUM with a scattered (b,j) AP (free) and writes the scan layout
   in 32B runs; the scattered-write ordering costs 2x (measured).
"""

import numpy as np

N, M, D = 1024, 8, 128
MARGIN = 0.3
EPS = 1e-12
NCORES = 8
A = N // NCORES          # anchors per core
CB = 256                 # column-batch size
NB = N // CB             # batches
G = M + 1                # scan group: 1 dummy + 8 j-steps
BIG = 1e9
SQ_BIAS = 2.0 + 4e-4

_CACHE = {}


def _build_nc():
    import concourse.bacc as bacc
    import concourse.mybir as mybir
    import concourse.tile as tile
    from concourse.tile import add_dep_helper

    fp32 = mybir.dt.float32
    f32r = mybir.dt.float32r
    AF = mybir.ActivationFunctionType
    OP = mybir.AluOpType
    AX = mybir.AxisListType

    nc = bacc.Bacc("TRN2", target_bir_lowering=False, debug=False,
                   num_devices=NCORES)

    x_in = nc.dram_tensor("x", [N * M, D], fp32, kind="ExternalInput")
    xa_in = nc.dram_tensor("xa", [A * M, D], fp32, kind="ExternalInput")
    pm_in = nc.dram_tensor("posmask", [A, N], fp32, kind="ExternalInput")
    nm_in = nc.dram_tensor("negmask", [A, N], fp32, kind="ExternalInput")
    out_t = nc.dram_tensor("row_loss", [A, 1], fp32, kind="ExternalOutput")

    S = (N * M) // 128   # 64 row-tiles of x
    SA = (A * M) // 128  # 8 row-tiles of xa

    with tile.TileContext(nc) as tc:
        with tc.tile_pool(name="persist", bufs=1) as persist:
            xT = persist.tile([128, N, M], f32r)    # x_hat^T, [d][b][j] (b-major)
            xTa = persist.tile([128, M, A], f32r)   # -2*x_hat_anchor^T, [d][i][a]
            pmask = persist.tile([128, N], fp32)
            nmask = persist.tile([128, N], fp32)
            up0 = persist.tile([128, CB * G], fp32)
            biasT = persist.tile([128, 1], fp32)
            apacc = persist.tile([128, NB], fp32)
            anacc = persist.tile([128, NB], fp32)
            RSLOT = 10
            arena = persist.tile([128, RSLOT, CB, G], fp32)

            nc.sync.dma_start(pmask[:], pm_in[:])
            nc.sync.dma_start(nmask[:], nm_in[:])
            nc.gpsimd.memset(biasT[:], SQ_BIAS)
            up0v = up0.rearrange("p (c g) -> p c g", g=G)
            nc.gpsimd.memset(up0v[:, :, 0:1], -BIG)
            nc.gpsimd.memset(up0v[:, :, 1:G], BIG)
            nc.gpsimd.memset(arena[:, :, :, 0:1], BIG)

            # ---------- setup: normalize + transpose (scale fused via diag) ----
            with (
                tc.tile_pool(name="setup", bufs=1) as setup,
                tc.tile_pool(name="chunk", bufs=2) as chunk,
                tc.tile_pool(name="dgp", bufs=3) as dgp,
                tc.tile_pool(name="tpsum", bufs=2, space="PSUM") as tpsum,
            ):
                def norm_rn(src_dram, n_tiles, neg2, tagp):
                    """Load rows (p-outer: row r = p*n_tiles + s); return
                    (xr, rn) with rn = scale/(||row||+eps)."""
                    xr = setup.tile([128, n_tiles, D], fp32, tag=f"xr{tagp}")
                    nc.sync.dma_start(
                        xr[:], src_dram.rearrange("(p s) d -> p s d", p=128))
                    n2 = setup.tile([128, n_tiles], fp32, tag=f"n2{tagp}")
                    CH = min(16, n_tiles)
                    for g in range(0, n_tiles, CH):
                        x2 = chunk.tile([128, 16, D], fp32, tag="x2c")
                        nc.scalar.activation(
                            x2[:, :CH, :], xr[:, g:g + CH, :], AF.Square)
                        nc.vector.tensor_reduce(
                            n2[:, g:g + CH], x2[:, :CH, :], axis=AX.X, op=OP.add)
                    nrm = setup.tile([128, n_tiles], fp32, tag=f"nr{tagp}")
                    nc.scalar.activation(nrm[:], n2[:], AF.Sqrt)
                    nc.vector.tensor_scalar_add(nrm[:], nrm[:], EPS)
                    rn = setup.tile([128, n_tiles], fp32, tag=f"rn{tagp}")
                    nc.vector.reciprocal(rn[:], nrm[:])
                    if neg2:
                        nc.vector.tensor_scalar_mul(rn[:], rn[:], -2.0)
                    return xr, rn

                def diag4(rn, s0, step4):
                    """diag(rn[:, s]) for s in s0..s0+3 as one [128, 4, 128]."""
                    dgc = dgp.tile([128, 4, 128], fp32, tag="dgc")
                    for jj in range(4):
                        nc.gpsimd.affine_select(
                            out=dgc[:, jj, :],
                            in_=rn[:, s0 + jj:s0 + jj + 1].to_broadcast((128, 128)),
                            compare_op=OP.is_equal, fill=0.0, base=0,
                            pattern=[[-1, 128]], channel_multiplier=1)
                    return dgc

                xr, rn = norm_rn(x_in, S, neg2=False, tagp="x")
                # tiles s = 8q + j hold rows r = p*64 + 8q + j -> (b = 8p+q, j)
                for q in range(M):
                    for half in range(2):
                        dgc = diag4(rn, 8 * q + 4 * half, 4)
                        pt = tpsum.tile([128, 4, 128], fp32, tag="tp")
                        for jj in range(4):
                            s = 8 * q + 4 * half + jj
                            nc.tensor.matmul(
                                pt[:, jj, :], lhsT=xr[:, s, :],
                                rhs=dgc[:, jj, :], start=True, stop=True)
                        dst = xT[:, q::M, 4 * half:4 * half + 4]
                        nc.scalar.activation(
                            dst.rearrange("d b j -> d j b"), pt[:], AF.Copy)

                xra, rna = norm_rn(xa_in, SA, neg2=True, tagp="a")
                for half in range(2):
                    dgc = diag4(rna, 4 * half, 4)
                    pt = tpsum.tile([128, 4, 128], fp32, tag="tp")
                    for jj in range(4):
                        s = 4 * half + jj
                        nc.tensor.matmul(
                            pt[:, jj, :], lhsT=xra[:, s, :],
                            rhs=dgc[:, jj, :], start=True, stop=True)
                    # tile s holds rows r = p*8+s -> (a=p, i=s)
                    dst = xTa[:, 4 * half:4 * half + 4, :]
                    nc.scalar.activation(dst, pt[:], AF.Copy)

            # ---------- main loop ----------
            with (
                tc.tile_pool(name="valsp", bufs=3) as valsp,
                tc.tile_pool(name="mtmp", bufs=6) as mtmp,
                tc.tile_pool(name="mpsum", bufs=2, space="PSUM") as mpsum,
            ):
                prev_tanh_last = None
                slot = 0
                for n in range(NB):
                    sd = []
                    sqrt_insts = []
                    for i in range(M):
                        pp = mpsum.tile([128, M, CB], fp32, tag="pp")
                        for j in range(M):
                            nc.tensor.matmul(
                                pp[:, j, :], lhsT=xTa[:, i, :],
                                rhs=xT[:, n * CB:(n + 1) * CB, j],
                                start=True, stop=True)
                        buf = arena[:, slot, :, :]
                        slot = (slot + 1) % RSLOT
                        # read PSUM scattered in (b, j) order; write 32B runs
                        inst = nc.scalar.activation(
                            buf[:, :, 1:G], pp.rearrange("p j b -> p b j"),
                            AF.Sqrt, bias=biasT[:, 0:1])
                        if prev_tanh_last is not None:
                            add_dep_helper(inst.ins, prev_tanh_last.ins,
                                           sync=False,
                                           reason="ACT table batch order")
                        sqrt_insts.append(inst)
                        sd.append(buf)
                    tanh_last = None
                    for i in range(M):
                        v = sd[i][:, :, 1:G]
                        t_inst = nc.scalar.activation(v, v, AF.Tanh, scale=0.5)
                        add_dep_helper(t_inst.ins, sqrt_insts[-1].ins,
                                       sync=False,
                                       reason="ACT table batch order")
                        tanh_last = t_inst
                    prev_tanh_last = tanh_last
                    prev_vals = None
                    for i in range(M):
                        vt = valsp.tile([128, CB * G], fp32, tag="vals")
                        d0 = up0[:] if i == 0 else prev_vals[:]
                        nc.vector.tensor_tensor_scan(
                            vt[:], d0, sd[i].rearrange("p c g -> p (c g)"),
                            0.0, OP.min, OP.add)
                        prev_vals = vt
                    dtw = prev_vals.rearrange("p (c g) -> p c g", g=G)[:, :, M:M + 1]
                    dtw = dtw.rearrange("p c o -> p (c o)")
                    tp = mtmp.tile([128, CB], fp32, tag="tp")
                    nc.vector.tensor_tensor(
                        tp[:], dtw, pmask[:, n * CB:(n + 1) * CB], OP.add)
                    nc.vector.tensor_reduce(
                        apacc[:, n:n + 1], tp[:], axis=AX.X, op=OP.max)
                    tn = mtmp.tile([128, CB], fp32, tag="tn")
                    nc.vector.tensor_tensor(
                        tn[:], dtw, nmask[:, n * CB:(n + 1) * CB], OP.add)
                    nc.vector.tensor_reduce(
                        anacc[:, n:n + 1], tn[:], axis=AX.X, op=OP.min)

                ap = mtmp.tile([128, 1], fp32, tag="fin")
                an = mtmp.tile([128, 1], fp32, tag="fin2")
                nc.vector.tensor_reduce(ap[:], apacc[:], axis=AX.X, op=OP.max)
                nc.vector.tensor_reduce(an[:], anacc[:], axis=AX.X, op=OP.min)
                lv = mtmp.tile([128, 1], fp32, tag="fin3")
                nc.vector.tensor_tensor(lv[:], ap[:], an[:], OP.subtract)
                nc.vector.tensor_scalar(
                    lv[:], lv[:], MARGIN, 0.0, OP.add, OP.max)
                nc.sync.dma_start(out_t[:], lv[:])

    nc.compile()
    return nc


def _get_nc():
    if "nc" not in _CACHE:
        _CACHE["nc"] = _build_nc()
    return _CACHE["nc"]


def kernel(inputs, labels, _trace=False, _trace_cores=None):
    from concourse.bass_utils import run_bass_kernel_spmd

    x = np.ascontiguousarray(np.asarray(inputs, dtype=np.float32)).reshape(N * M, D)
    lab = np.asarray(labels)
    eq = lab[:, None] == lab[None, :]
    posmask = np.where(eq, np.float32(0.0), np.float32(-1e30)).astype(np.float32)
    negmask = np.where(eq, np.float32(1e30), np.float32(0.0)).astype(np.float32)

    nc = _get_nc()
    in_maps = []
    for c in range(NCORES):
        a0 = c * A
        in_maps.append({
            "x": x,
            "xa": np.ascontiguousarray(x[a0 * M:(a0 + A) * M]),
            "posmask": np.ascontiguousarray(posmask[a0:a0 + A]),
            "negmask": np.ascontiguousarray(negmask[a0:a0 + A]),
        })
    res = run_bass_kernel_spmd(
        nc, in_maps, core_ids=list(range(NCORES)), trace=_trace,
        trace_cores=_trace_cores)
    if _trace:
        _CACHE["last_results"] = res
    row_loss = np.concatenate([r["row_loss"][:, 0] for r in res.results])
    return np.asarray(row_loss.mean(), dtype=np.float32)
